# revision 41
# baseline (speedup 1.0000x reference)
"""Windowed sparse attention (16x16 windows, keys from x+skip) on 8 TRN2 NeuronCores.

Reference computation (all 1x1 convs + per-window attention):
  q = Wq @ x;  k,v = split(Wkv @ [x;skip]);  per 16x16 window w/ 256 queries and
  512 keys (256 from x, 256 from skip):  out = softmax(q k^T / 8) v;  y = Wo @ out + bo.

Sharding: each core takes one 16-row strip of the 128x128 image (one window-row X),
both batch elements — all 128 of its windows are fully local; only weights replicated.

Per-core dataflow (fp32r matmuls at full PE rate, transposed-softmax layout):
  - projections produce qT/kT [d, pixels] and v in [pixel, (h d)] layout directly
  - simT[j,i] = kT^T @ qT per window; exp on ScalarE (scale=1/8 folded in)
  - attn@v via lhsT = [v | ones]: softmax denominator s arrives free as psum row 64
  - recip(s) on DVE per window-pair, broadcast via DRAM-bounce DMA,
    normalize-mult on GPSIMD
  - Wo projection from head-stacked normalized outputs; bias added via a
    host-precomputed broadcast tile during the DVE evacuation
  - PSUM->SBUF evacuations split between DVE and ScalarE to balance engines
"""
import sys

if '/opt/trn_rl_repo' not in sys.path:
    sys.path.insert(0, '/opt/trn_rl_repo')

import numpy as np
import ml_dtypes
import concourse.bass as bass
import concourse.tile as tile
import concourse.mybir as mybir
from concourse.bass_utils import run_bass_kernel_spmd

F32 = mybir.dt.float32
F32R = mybir.dt.float32r
BF16 = mybir.dt.bfloat16
AFT = mybir.ActivationFunctionType

N_CORES = 8
B = 2            # batch
C = 256          # model channels
H = 8            # heads
D = 64           # head dim
INNER = H * D    # 512
WIN = 16         # window side
RS = 16          # strip rows per core (= one window row)
WCOL = 128       # image width
PX = RS * WCOL   # 2048 pixels per (batch, strip)
NY = 8           # windows along width
NI = WIN * WIN   # 256 queries per window
BODIES_PER_ITER = 2   # bodies per For_i iteration (amortizes barrier + drain)


def _split_multiwaits(nc, max_waits=1):
    """walrus codegen rejects instructions carrying >1 sem wait (seen on the
    TileContext exit drain); hoist extras onto single-wait NoOps just before."""
    for f in nc.m.functions:
        for blk in f.blocks:
            out, changed = [], False
            for ins in blk.instructions:
                si = ins.sync_info
                if si is not None and len(si.on_wait) > max_waits:
                    waits = list(si.on_wait)
                    SyncInfo = type(si)
                    extra, keep = waits[:-max_waits], waits[-max_waits:]
                    for i, w in enumerate(extra):
                        n = mybir.InstNoOp(name=f"{ins.name}-sw{i}", ins=[], outs=[])
                        n.engine = ins.engine
                        n.sync_info = SyncInfo(on_wait=[w], on_update=[])
                        out.append(n)
                    si.on_wait = keep
                    changed = True
                out.append(ins)
            if changed:
                blk.instructions = out
    return nc


class Ctx:
    pass


def _emit_projections(g, b, y0, strips):
    """Project q/k/v for the window PAIR (y0, y0+1) (generator), so the
    caller can weave these PE ops into attention's exp-latency gaps.
    Strips are window-major: the pair is a contiguous 512-px slice; q/k
    matmuls run at N=512 over both windows (half the instructions)."""
    nc = g.nc
    NI2 = 2 * NI
    xw = [strips[b][pi][:, :, y0 * NI:(y0 + 2) * NI] for pi in range(2)]

    # q: qT [128(2h), 2 win, NI] x 4 head-pair chunks; pair-batched matmuls
    q_p = [g.qkpool.tile([128, 4, NI], BF16, name=f"q{b}_{y0}_{t}", tag=f"q{t}")
           for t in range(2)]
    for mc2 in range(2):
        pq = g.psim.tile([128, 2, NI2], F32, tag="simpa", name=f"pq{b}_{y0}_{mc2}")
        for mi in range(2):
            mc = mc2 * 2 + mi
            for kc in range(2):
                nc.tensor.matmul(
                    pq[:, mi], g.wq_sb[:, kc, mc * 128:(mc + 1) * 128],
                    xw[0][:, kc], start=(kc == 0), stop=(kc == 1))
        for t in range(2):
            nc.vector.tensor_copy(q_p[t][:, mc2 * 2:(mc2 + 1) * 2],
                                  pq[:, :, t * NI:(t + 1) * NI])
        yield None

    # k: kT [128(2h), 4 kc4, 2 part, NI] per window x 4 chunks
    k_p = [g.qkpool.tile([128, 4, 2, NI], BF16, name=f"k{b}_{y0}_{t}", tag=f"k{t}")
           for t in range(2)]
    for kc4 in range(4):
        pk = g.psim.tile([128, 2, NI2], F32, tag="simpa", name=f"pk{b}_{y0}_{kc4}")
        for pi in range(2):
            for kc in range(2):
                nc.tensor.matmul(
                    pk[:, pi], g.wk_sb[:, kc, kc4 * 128:(kc4 + 1) * 128],
                    xw[pi][:, kc], start=(kc == 0), stop=(kc == 1))
        for t in range(2):
            nc.vector.tensor_copy(k_p[t][:, kc4], pk[:, :, t * NI:(t + 1) * NI])
        if kc4 % 2 == 1:
            yield None

    # v (transposed): per window one tile [128 j, 4 chunks, 8h x (64|ones)];
    # single strided ones-write per window covers all chunks/heads.
    v_p = []
    for t in range(2):
        v_y = g.vpool.tile([128, 4, H, D + 1], F32R, name=f"v{b}_{y0}_{t}", tag="v")
        nc.vector.tensor_copy(v_y[:, :, :, D],
                              nc.const_aps.tensor(1.0, (128, 4 * H), F32))
        for pi_jc in range(4):
            pi, jc = pi_jc // 2, pi_jc % 2
            pv = g.pproj.tile([128, INNER], F32, tag="pp")
            for kc in range(2):
                nc.tensor.matmul(
                    pv[:], xw[pi][:, kc, t * NI + jc * 128:t * NI + (jc + 1) * 128],
                    g.wv_sb[:, kc, :], start=(kc == 0), stop=(kc == 1))
            if pi_jc % 2 == 0:
                nc.scalar.copy(v_y[:, pi_jc, :, 0:D],
                               pv[:].rearrange("p (h d) -> p h d", h=H))
            else:
                nc.vector.tensor_copy(
                    v_y[:, pi_jc, :, 0:D], pv[:].rearrange("p (h d) -> p h d", h=H))
        v_p.append(v_y)
        if t == 0:
            yield None
    yield (q_p, k_p, v_p)


def _emit_attention(g, b, y, q_y, k_y, v_y, filler=None):
    """simT -> exp -> (attn@[v|1]) for all 8 heads; returns unnorm [65, H, NI].

    The av matmuls for head-pair hp are emitted after the sim matmuls of
    hp+1, so ScalarE's exp latency hides under PE work instead of stalling
    the in-order PE stream. sim and av PSUM tiles share one pool (3 slots of
    2 banks) to stay inside the 8-bank budget."""
    nc = g.nc
    un = g.unpool.tile([65, H, NI], F32, name=f"u{b}_{y}", tag="un")

    def emit_sim(hp):
        sims = [g.psim.tile([128, 4, NI], F32, tag="simpa",
                            name=f"sim{b}_{y}_{hp}_{h2}") for h2 in range(2)]
        for pi in range(2):
            for jh in range(2):
                for h2 in range(2):
                    nc.tensor.matmul(
                        sims[h2][:, pi * 2 + jh],
                        k_y[h2 * 64:(h2 + 1) * 64, hp, pi, jh * 128:(jh + 1) * 128],
                        q_y[h2 * 64:(h2 + 1) * 64, hp],
                        start=True, stop=True, tile_position=(h2 * 64, 0))
        exs = []
        for h2 in range(2):
            ex = g.expool.tile([128, 2, 2, NI], F32R, tag="ex")
            nc.scalar.activation(ex[:], sims[h2][:], AFT.Exp, scale=0.125)
            exs.append(ex)
        return exs

    def emit_av(hp, exs):
        pa = g.psim.tile([65, 2, NI], F32, tag="simpa", name=f"pa{b}_{y}_{hp}")
        for h2 in range(2):
            h = hp * 2 + h2
            for n_mm, (pi, jc) in enumerate([(0, 0), (0, 1), (1, 0), (1, 1)]):
                nc.tensor.matmul(
                    pa[:, h2], v_y[:, pi * 2 + jc, h, :], exs[h2][:, pi, jc],
                    start=(n_mm == 0), stop=(n_mm == 3))
        # evacuate: hp 0-2 on DVE, hp 3 on ScalarE (engine balance)
        if hp < 3:
            nc.vector.tensor_copy(un[:, hp * 2:hp * 2 + 2], pa[:])
        else:
            nc.scalar.copy(un[:, hp * 2:hp * 2 + 2], pa[:])

    prev = None
    for hp in range(4):
        exs = emit_sim(hp)
        if prev is not None:
            emit_av(prev[0], prev[1])
        if filler is not None:
            filler()
        prev = (hp, exs)
    emit_av(prev[0], prev[1])
    if filler is not None:
        filler()
    return un


def _emit_norm_chain(g, b, ys, uns):
    """Stage 1 for a finished pair: recip(s), broadcast, GPSIMD normalize."""
    nc = g.nc
    s_pair = g.spool.tile([2 * H, NI], F32, name=f"sm{b}_{ys[0]}", tag="sm")
    for t, yy in enumerate(ys):
        nc.sync.dma_start(s_pair[t * H:(t + 1) * H, :], uns[t][64:65])
    s_rec = g.spool.tile([2 * H, NI], F32, name=f"sr{b}_{ys[0]}", tag="sr")
    nc.vector.reciprocal(s_rec[:], s_pair[:])
    s_rb = g.spool.tile([2 * H, NI], BF16, name=f"sb{b}_{ys[0]}", tag="sb")
    nc.vector.tensor_copy(s_rb[:], s_rec[:])
    s_dram = g.dpool.tile([2 * H, NI], BF16, name=f"sd{b}_{ys[0]}", tag="sd")
    nc.sync.dma_start(s_dram[:], s_rb[:])

    # one broadcast DMA for the whole pair: [64 dup, 2 win, H, NI]
    bc = g.bcpool.tile([64, 2, H, NI], BF16, name=f"bc{b}_{ys[0]}", tag="bc")
    APcls = type(bc[:])
    row = s_dram[0]
    nc.sync.dma_start(
        bc[:], APcls(tensor=row.tensor, offset=row.offset,
                     ap=[[0, 64], [NI, 2 * H], [1, NI]]))

    atts = []
    for t, yy in enumerate(ys):
        un = uns[t]
        att = g.atpool.tile([128, 4, NI], BF16, name=f"at{b}_{yy}", tag="at")
        odd = g.atpool.tile([64, 4, NI], BF16, name=f"od{b}_{yy}", tag="od", bufs=1)
        # even heads -> partitions 0..63; odd heads -> temp, DMA to 64..127
        nc.gpsimd.tensor_tensor(att[0:64], un[0:64, 0:H:2], bc[:, t, 0:H:2],
                                mybir.AluOpType.mult)
        nc.gpsimd.tensor_tensor(odd[:], un[0:64, 1:H:2], bc[:, t, 1:H:2],
                                mybir.AluOpType.mult)
        nc.sync.dma_start(att[64:128], odd[:])
        atts.append(att)
    return atts


def _emit_wo(g, b, ys, atts):
    """Stage 2: Wo projection + bias + store for a normalized pair."""
    nc = g.nc
    for att, yy in zip(atts, ys):
        po = g.pproj.tile([128, 2, NI], F32, tag="pp", name=f"po{b}_{yy}")
        for mc in range(2):
            for hp in range(4):
                nc.tensor.matmul(po[:, mc], g.wo_sb[:, hp, mc * 128:(mc + 1) * 128],
                                 att[:, hp], start=(hp == 0), stop=(hp == 3))
        ot = g.outpool.tile([128, 2, NI], BF16, name=f"ot{b}_{yy}", tag="ot")
        nc.vector.tensor_tensor(ot[:], po[:], g.bias_bc[:], mybir.AluOpType.add)
        dst = g.out_d[b].rearrange("(mc p) r w -> p mc (r w)", p=128)
        nc.sync.dma_start(dst[:, :, yy * NI:(yy + 1) * NI], ot[:])


def build_program(reps=1, phases=3, split_mw=True):
    nc = bass.Bass("TRN2", target_bir_lowering=False, debug=False, num_devices=N_CORES)
    g = Ctx()
    g.nc = nc

    x_d = nc.dram_tensor("x", [B, C, RS, WCOL], BF16, kind="ExternalInput").ap()
    s_d = nc.dram_tensor("skip", [B, C, RS, WCOL], BF16, kind="ExternalInput").ap()
    wq_d = nc.dram_tensor("wqT", [C, INNER], BF16, kind="ExternalInput").ap()
    wk_d = nc.dram_tensor("wkT", [C, INNER], BF16, kind="ExternalInput").ap()
    wv_d = nc.dram_tensor("wvT", [C, INNER], BF16, kind="ExternalInput").ap()
    wo_d = nc.dram_tensor("woT", [INNER, C], BF16, kind="ExternalInput").ap()
    bb_d = nc.dram_tensor("bias_bc", [128, 2, NI], F32, kind="ExternalInput").ap()
    g.out_d = nc.dram_tensor("out", [B, C, RS, WCOL], BF16, kind="ExternalOutput").ap()

    with tile.TileContext(nc) as tc:
        with (
            tc.tile_pool(name="wpool", bufs=1) as wpool,
            tc.tile_pool(name="xpool", bufs=2) as xpool,
            tc.tile_pool(name="qk", bufs=2) as qkpool,
            tc.tile_pool(name="vp", bufs=2) as vpool,
            tc.tile_pool(name="ex", bufs=3) as expool,
            tc.tile_pool(name="un", bufs=4) as unpool,
            tc.tile_pool(name="at", bufs=3) as atpool,
            tc.tile_pool(name="bc", bufs=1) as bcpool,
            tc.tile_pool(name="sp", bufs=1) as spool,
            tc.tile_pool(name="ou", bufs=2) as outpool,
            tc.tile_pool(name="pproj", bufs=2, space="PSUM") as pproj,
            tc.tile_pool(name="psim", bufs=3, space="PSUM") as psim,
            tc.tile_pool(name="dram", bufs=3, space="DRAM") as dpool,
        ):
            g.qkpool, g.vpool, g.expool = qkpool, vpool, expool
            g.unpool, g.atpool, g.bcpool, g.spool = unpool, atpool, bcpool, spool
            g.outpool, g.pproj, g.psim, g.dpool = outpool, pproj, psim, dpool

            g.wq_sb = wpool.tile([128, 2, INNER], BF16, name="wq")
            nc.sync.dma_start(g.wq_sb[:], wq_d.rearrange("(kc p) m -> p kc m", p=128))
            g.wk_sb = wpool.tile([128, 2, INNER], BF16, name="wk")
            nc.sync.dma_start(g.wk_sb[:], wk_d.rearrange("(kc p) m -> p kc m", p=128))
            g.wv_sb = wpool.tile([128, 2, INNER], BF16, name="wv")
            nc.sync.dma_start(g.wv_sb[:], wv_d.rearrange("(kc p) m -> p kc m", p=128))
            g.wo_sb = wpool.tile([128, 4, C], BF16, name="wo")
            nc.sync.dma_start(g.wo_sb[:], wo_d.rearrange("(kc p) m -> p kc m", p=128))
            g.bias_bc = wpool.tile([128, 2, NI], F32, name="bb")
            nc.sync.dma_start(g.bias_bc[:], bb_d)

            # normalize/Wo queues are carried between the two bodies of one
            # For_i iteration (same block), and fully drained at iteration
            # end — loop iterations stay self-contained.
            state = {"chain": [], "wo": []}

            def _body(drain, prefetch_next=False):
                _emit_windows(g, state, x_d, s_d, xpool, phases, drain=drain,
                              prefetch_next=prefetch_next)

            if reps == 1:
                _body(True)
            elif reps < 0:
                with tc.For_i(0, -reps, 1):   # BODIES_PER_ITER bodies/iteration
                    for bi in range(BODIES_PER_ITER):
                        last = bi == BODIES_PER_ITER - 1
                        _body(last, prefetch_next=not last)
            else:
                with tc.For_i(0, reps, 1):
                    _body(True)

    if split_mw:
        _split_multiwaits(nc)
    return nc


def _load_strips(g, b, x_d, s_d, xpool):
    """Load one batch's x/skip strip. Split into window-range chunks so the
    first windows' data lands quickly after an iteration barrier."""
    nc = g.nc
    x_sb = xpool.tile([128, 2, PX], BF16, name=f"x{b}", tag="xs")
    sk_sb = xpool.tile([128, 2, PX], BF16, name=f"s{b}", tag="ss")
    xr = x_d[b].rearrange("(kc p) r w -> p kc (r w)", p=128)
    sr = s_d[b].rearrange("(kc p) r w -> p kc (r w)", p=128)
    cuts = (0, 512, 2048)
    for c0, c1 in zip(cuts[:-1], cuts[1:]):
        nc.sync.dma_start(x_sb[:, :, c0:c1], xr[:, :, c0:c1])
        nc.sync.dma_start(sk_sb[:, :, c0:c1], sr[:, :, c0:c1])
    return (x_sb, sk_sb)


def _emit_windows(g, state, x_d, s_d, xpool, phases=3, drain=True,
                  prefetch_next=False):
    """One body: all 16 (b, y) windows flat. The normalize/Wo queues defer
    their tail into the following body when drain=False (two bodies per
    For_i iteration share one block), else drain fully. With prefetch_next,
    the next body's strips are loaded mid-body (after their last consumer)."""
    nc = g.nc
    strips = state.pop("next_strips", None)
    if strips is None:
        strips = [_load_strips(g, b, x_d, s_d, xpool) for b in range(B)]
    pairs = [(b, y0) for b in range(B) for y0 in range(0, NY, 2)]

    def drain_gen(gen):
        out = None
        for out in gen:
            pass
        return out

    cur = drain_gen(_emit_projections(g, *pairs[0], strips))
    for pidx, (b, y0) in enumerate(pairs):
        if phases < 2:
            if pidx + 1 < len(pairs):
                cur = drain_gen(_emit_projections(g, *pairs[pidx + 1], strips))
            continue
        nxt_gen = (_emit_projections(g, *pairs[pidx + 1], strips)
                   if pidx + 1 < len(pairs) else None)
        nxt_result = [None]
        def filler():
            if nxt_gen is not None:
                r = next(nxt_gen, None)
                if r is not None:
                    nxt_result[0] = r
        q_p, k_p, v_p = cur
        uns = [_emit_attention(g, b, y0 + t, q_p[t], k_p[t], v_p[t], filler=filler)
               for t in range(2)]
        if nxt_gen is not None:
            r = drain_gen(nxt_gen)
            if r is not None:
                nxt_result[0] = r
            cur = nxt_result[0]
        if phases < 3:
            continue
        state["chain"].append((b, (y0, y0 + 1), uns))
        if len(state["chain"]) > 1:
            cb, cys, cuns = state["chain"].pop(0)
            state["wo"].append((cb, cys, _emit_norm_chain(g, cb, cys, cuns)))
        if len(state["wo"]) > 1:
            wb, wys, atts = state["wo"].pop(0)
            _emit_wo(g, wb, wys, atts)
        if prefetch_next:
            if pidx == 4:
                state.setdefault("next_strips", [None, None])[0] = \
                    _load_strips(g, 0, x_d, s_d, xpool)
            elif pidx == 6:
                state["next_strips"][1] = _load_strips(g, 1, x_d, s_d, xpool)
    if drain:
        for cb, cys, cuns in state["chain"]:
            state["wo"].append((cb, cys, _emit_norm_chain(g, cb, cys, cuns)))
        state["chain"] = []
        for wb, wys, atts in state["wo"]:
            _emit_wo(g, wb, wys, atts)
        state["wo"] = []


_PROGRAM = None


def _get_program():
    global _PROGRAM
    if _PROGRAM is None:
        _PROGRAM = build_program()
    return _PROGRAM


def _make_bias_bc(bo):
    # [C] -> [128 p, 2 mc, NI] broadcast along NI (channel = mc*128 + p)
    bb = np.asarray(bo, np.float32).reshape(2, 128).transpose(1, 0)
    return np.ascontiguousarray(np.broadcast_to(bb[:, :, None], (128, 2, NI)).copy())


def kernel(x, skip, Wq, Wkv, Wo, bo):
    bf = ml_dtypes.bfloat16
    x = np.asarray(x, dtype=np.float32)
    skip = np.asarray(skip, dtype=np.float32)
    wqT = np.ascontiguousarray(np.asarray(Wq, np.float32).T).astype(bf)
    wkT = np.ascontiguousarray(np.asarray(Wkv, np.float32)[:INNER].T).astype(bf)
    wvT = np.ascontiguousarray(np.asarray(Wkv, np.float32)[INNER:].T).astype(bf)
    woT = np.ascontiguousarray(np.asarray(Wo, np.float32).T).astype(bf)
    bias_bc = _make_bias_bc(bo)

    nc = _get_program()
    in_maps = []
    for c in range(N_CORES):
        r0, r1 = c * RS, (c + 1) * RS
        in_maps.append({
            "x": _to_window_major(x[:, :, r0:r1, :]).astype(bf),
            "skip": _to_window_major(skip[:, :, r0:r1, :]).astype(bf),
            "wqT": wqT, "wkT": wkT, "wvT": wvT, "woT": woT,
            "bias_bc": bias_bc,
        })
    res = run_bass_kernel_spmd(nc, in_maps, list(range(N_CORES)))
    out = np.empty((B, C, N_CORES * RS, WCOL), dtype=np.float32)
    for c in range(N_CORES):
        out[:, :, c * RS:(c + 1) * RS, :] = _from_window_major(
            np.asarray(res.results[c]["out"]).astype(np.float32))
    return out


def _to_window_major(strip):
    # [B, C, 16, 128] row-major pixels -> pixel axis reordered to (y, r, c)
    s = strip.reshape(B, C, RS, NY, WIN).transpose(0, 1, 3, 2, 4)
    return np.ascontiguousarray(s).reshape(B, C, RS, WCOL)


def _from_window_major(strip):
    # inverse of _to_window_major
    s = strip.reshape(B, C, NY, RS, WIN).transpose(0, 1, 3, 2, 4)
    return np.ascontiguousarray(s).reshape(B, C, RS, WCOL)


# revision 43
# speedup vs baseline: 1.1427x; 1.1427x over previous
"""Windowed sparse attention (16x16 windows, keys from x+skip) on 8 TRN2 NeuronCores.

Reference computation (all 1x1 convs + per-window attention):
  q = Wq @ x;  k,v = split(Wkv @ [x;skip]);  per 16x16 window w/ 256 queries and
  512 keys (256 from x, 256 from skip):  out = softmax(q k^T / 8) v;  y = Wo @ out + bo.

Sharding: each core takes one 16-row strip of the 128x128 image (one window-row X),
both batch elements — all 128 of its windows are fully local; only weights replicated.

Per-core dataflow (fp32r matmuls at full PE rate, transposed-softmax layout):
  - projections produce qT/kT [d, pixels] and v in [pixel, (h d)] layout directly
  - simT[j,i] = kT^T @ qT per window; exp on ScalarE (scale=1/8 folded in)
  - attn@v via lhsT = [v | ones]: softmax denominator s arrives free as psum row 64
  - recip(s) on DVE per window-pair, broadcast via DRAM-bounce DMA,
    normalize-mult on GPSIMD
  - Wo projection from head-stacked normalized outputs; bias added via a
    host-precomputed broadcast tile during the DVE evacuation
  - PSUM->SBUF evacuations split between DVE and ScalarE to balance engines
"""
import sys

if '/opt/trn_rl_repo' not in sys.path:
    sys.path.insert(0, '/opt/trn_rl_repo')

import numpy as np
import ml_dtypes
import concourse.bass as bass
import concourse.tile as tile
import concourse.mybir as mybir
from concourse.bass_utils import run_bass_kernel_spmd

F32 = mybir.dt.float32
F32R = mybir.dt.float32r
BF16 = mybir.dt.bfloat16
AFT = mybir.ActivationFunctionType

N_CORES = 8
B = 2            # batch
C = 256          # model channels
H = 8            # heads
D = 64           # head dim
INNER = H * D    # 512
WIN = 16         # window side
RS = 16          # strip rows per core (= one window row)
WCOL = 128       # image width
PX = RS * WCOL   # 2048 pixels per (batch, strip)
NY = 8           # windows along width
NI = WIN * WIN   # 256 queries per window
BODIES_PER_ITER = 2   # bodies per For_i iteration (amortizes barrier + drain)


def _split_multiwaits(nc, max_waits=1):
    """walrus codegen rejects instructions carrying >1 sem wait (seen on the
    TileContext exit drain); hoist extras onto single-wait NoOps just before."""
    for f in nc.m.functions:
        for blk in f.blocks:
            out, changed = [], False
            for ins in blk.instructions:
                si = ins.sync_info
                if si is not None and len(si.on_wait) > max_waits:
                    waits = list(si.on_wait)
                    SyncInfo = type(si)
                    extra, keep = waits[:-max_waits], waits[-max_waits:]
                    for i, w in enumerate(extra):
                        n = mybir.InstNoOp(name=f"{ins.name}-sw{i}", ins=[], outs=[])
                        n.engine = ins.engine
                        n.sync_info = SyncInfo(on_wait=[w], on_update=[])
                        out.append(n)
                    si.on_wait = keep
                    changed = True
                out.append(ins)
            if changed:
                blk.instructions = out
    return nc


class Ctx:
    pass


def _emit_projections(g, b, y0, strips):
    """Project q/k/v for the window PAIR (y0, y0+1) (generator), so the
    caller can weave these PE ops into attention's exp-latency gaps.
    Strips are window-major: the pair is a contiguous 512-px slice; q/k
    matmuls run at N=512 over both windows (half the instructions)."""
    nc = g.nc
    NI2 = 2 * NI
    xw = [strips[b][pi][:, :, y0 * NI:(y0 + 2) * NI] for pi in range(2)]

    # q: qT [128(2h), 2 win, NI] x 4 head-pair chunks; pair-batched matmuls
    q_p = [g.qkpool.tile([128, 4, NI], BF16, name=f"q{b}_{y0}_{t}", tag=f"q{t}")
           for t in range(2)]
    for mc2 in range(2):
        pq = g.psim.tile([128, 2, NI2], F32, tag="simpa", name=f"pq{b}_{y0}_{mc2}")
        for mi in range(2):
            mc = mc2 * 2 + mi
            for kc in range(2):
                nc.tensor.matmul(
                    pq[:, mi], g.wq_sb[:, kc, mc * 128:(mc + 1) * 128],
                    xw[0][:, kc], start=(kc == 0), stop=(kc == 1))
        for t in range(2):
            nc.vector.tensor_copy(q_p[t][:, mc2 * 2:(mc2 + 1) * 2],
                                  pq[:, :, t * NI:(t + 1) * NI])
        yield None

    # k: kT [128(2h), 4 kc4, 2 part, NI] per window x 4 chunks
    k_p = [g.qkpool.tile([128, 4, 2, NI], BF16, name=f"k{b}_{y0}_{t}", tag=f"k{t}")
           for t in range(2)]
    for kc4 in range(4):
        pk = g.psim.tile([128, 2, NI2], F32, tag="simpa", name=f"pk{b}_{y0}_{kc4}")
        for pi in range(2):
            for kc in range(2):
                nc.tensor.matmul(
                    pk[:, pi], g.wk_sb[:, kc, kc4 * 128:(kc4 + 1) * 128],
                    xw[pi][:, kc], start=(kc == 0), stop=(kc == 1))
        for t in range(2):
            nc.vector.tensor_copy(k_p[t][:, kc4], pk[:, :, t * NI:(t + 1) * NI])
        if kc4 % 2 == 1:
            yield None

    # v (transposed): per window one tile [128 j, 4 chunks, 8h x (64|ones)];
    # single strided ones-write per window covers all chunks/heads.
    v_p = []
    for t in range(2):
        v_y = g.vpool.tile([128, 4, H, D + 1], BF16, name=f"v{b}_{y0}_{t}", tag="v")
        nc.vector.tensor_copy(v_y[:, :, :, D],
                              nc.const_aps.tensor(1.0, (128, 4 * H), F32))
        for pi_jc in range(4):
            pi, jc = pi_jc // 2, pi_jc % 2
            pv = g.pproj.tile([128, INNER], F32, tag="pp")
            for kc in range(2):
                nc.tensor.matmul(
                    pv[:], xw[pi][:, kc, t * NI + jc * 128:t * NI + (jc + 1) * 128],
                    g.wv_sb[:, kc, :], start=(kc == 0), stop=(kc == 1))
            if pi_jc % 2 == 0:
                nc.scalar.copy(v_y[:, pi_jc, :, 0:D],
                               pv[:].rearrange("p (h d) -> p h d", h=H))
            else:
                nc.vector.tensor_copy(
                    v_y[:, pi_jc, :, 0:D], pv[:].rearrange("p (h d) -> p h d", h=H))
        v_p.append(v_y)
        if t == 0:
            yield None
    yield (q_p, k_p, v_p)


def _emit_attention(g, b, y, q_y, k_y, v_y, filler=None):
    """simT -> exp -> (attn@[v|1]) for all 8 heads; returns unnorm [65, H, NI].

    The av matmuls for head-pair hp are emitted after the sim matmuls of
    hp+1, so ScalarE's exp latency hides under PE work instead of stalling
    the in-order PE stream. sim and av PSUM tiles share one pool (3 slots of
    2 banks) to stay inside the 8-bank budget."""
    nc = g.nc
    un = g.unpool.tile([65, H, NI], F32, name=f"u{b}_{y}", tag="un")

    def emit_sim(hp):
        sims = [g.psim.tile([128, 4, NI], F32, tag="simpa",
                            name=f"sim{b}_{y}_{hp}_{h2}") for h2 in range(2)]
        for pi in range(2):
            for jh in range(2):
                for h2 in range(2):
                    nc.tensor.matmul(
                        sims[h2][:, pi * 2 + jh],
                        k_y[h2 * 64:(h2 + 1) * 64, hp, pi, jh * 128:(jh + 1) * 128],
                        q_y[h2 * 64:(h2 + 1) * 64, hp],
                        start=True, stop=True, tile_position=(h2 * 64, 0))
        exs = []
        for h2 in range(2):
            ex = g.expool.tile([128, 2, 2, NI], BF16, tag="ex")
            nc.scalar.activation(ex[:], sims[h2][:], AFT.Exp, scale=0.125)
            exs.append(ex)
        return exs

    def emit_av(hp, exs):
        pa = g.psim.tile([65, 2, NI], F32, tag="simpa", name=f"pa{b}_{y}_{hp}")
        for h2 in range(2):
            h = hp * 2 + h2
            for n_mm, (pi, jc) in enumerate([(0, 0), (0, 1), (1, 0), (1, 1)]):
                nc.tensor.matmul(
                    pa[:, h2], v_y[:, pi * 2 + jc, h, :], exs[h2][:, pi, jc],
                    start=(n_mm == 0), stop=(n_mm == 3))
        # evacuate: hp 0-2 on DVE, hp 3 on ScalarE (engine balance)
        if hp < 3:
            nc.vector.tensor_copy(un[:, hp * 2:hp * 2 + 2], pa[:])
        else:
            nc.scalar.copy(un[:, hp * 2:hp * 2 + 2], pa[:])

    prev = None
    for hp in range(4):
        exs = emit_sim(hp)
        if prev is not None:
            emit_av(prev[0], prev[1])
        if filler is not None:
            filler()
        prev = (hp, exs)
    emit_av(prev[0], prev[1])
    if filler is not None:
        filler()
    return un


def _emit_norm_chain(g, b, ys, uns):
    """Stage 1 for a finished pair: recip(s), broadcast, GPSIMD normalize."""
    nc = g.nc
    s_pair = g.spool.tile([2 * H, NI], F32, name=f"sm{b}_{ys[0]}", tag="sm")
    for t, yy in enumerate(ys):
        nc.sync.dma_start(s_pair[t * H:(t + 1) * H, :], uns[t][64:65])
    s_rec = g.spool.tile([2 * H, NI], F32, name=f"sr{b}_{ys[0]}", tag="sr")
    nc.vector.reciprocal(s_rec[:], s_pair[:])
    s_rb = g.spool.tile([2 * H, NI], BF16, name=f"sb{b}_{ys[0]}", tag="sb")
    nc.vector.tensor_copy(s_rb[:], s_rec[:])
    s_dram = g.dpool.tile([2 * H, NI], BF16, name=f"sd{b}_{ys[0]}", tag="sd")
    nc.sync.dma_start(s_dram[:], s_rb[:])

    # one broadcast DMA for the whole pair: [64 dup, 2 win, H, NI]
    bc = g.bcpool.tile([64, 2, H, NI], BF16, name=f"bc{b}_{ys[0]}", tag="bc")
    APcls = type(bc[:])
    row = s_dram[0]
    nc.sync.dma_start(
        bc[:], APcls(tensor=row.tensor, offset=row.offset,
                     ap=[[0, 64], [NI, 2 * H], [1, NI]]))

    atts = []
    for t, yy in enumerate(ys):
        un = uns[t]
        att = g.atpool.tile([128, 4, NI], BF16, name=f"at{b}_{yy}", tag="at")
        odd = g.atpool.tile([64, 4, NI], BF16, name=f"od{b}_{yy}", tag="od", bufs=1)
        # even heads -> partitions 0..63; odd heads -> temp, DMA to 64..127
        nc.gpsimd.tensor_tensor(att[0:64], un[0:64, 0:H:2], bc[:, t, 0:H:2],
                                mybir.AluOpType.mult)
        nc.gpsimd.tensor_tensor(odd[:], un[0:64, 1:H:2], bc[:, t, 1:H:2],
                                mybir.AluOpType.mult)
        nc.sync.dma_start(att[64:128], odd[:])
        atts.append(att)
    return atts


def _emit_wo(g, b, ys, atts):
    """Stage 2: Wo projection + bias + store for a normalized pair."""
    nc = g.nc
    for att, yy in zip(atts, ys):
        po = g.pproj.tile([128, 2, NI], F32, tag="pp", name=f"po{b}_{yy}")
        for mc in range(2):
            for hp in range(4):
                nc.tensor.matmul(po[:, mc], g.wo_sb[:, hp, mc * 128:(mc + 1) * 128],
                                 att[:, hp], start=(hp == 0), stop=(hp == 3))
        ot = g.outpool.tile([128, 2, NI], BF16, name=f"ot{b}_{yy}", tag="ot")
        nc.vector.tensor_tensor(ot[:], po[:], g.bias_bc[:], mybir.AluOpType.add)
        dst = g.out_d[b].rearrange("(mc p) r w -> p mc (r w)", p=128)
        nc.sync.dma_start(dst[:, :, yy * NI:(yy + 1) * NI], ot[:])


def build_program(reps=1, phases=3, split_mw=True):
    nc = bass.Bass("TRN2", target_bir_lowering=False, debug=False, num_devices=N_CORES)
    g = Ctx()
    g.nc = nc

    x_d = nc.dram_tensor("x", [B, C, RS, WCOL], BF16, kind="ExternalInput").ap()
    s_d = nc.dram_tensor("skip", [B, C, RS, WCOL], BF16, kind="ExternalInput").ap()
    wq_d = nc.dram_tensor("wqT", [C, INNER], BF16, kind="ExternalInput").ap()
    wk_d = nc.dram_tensor("wkT", [C, INNER], BF16, kind="ExternalInput").ap()
    wv_d = nc.dram_tensor("wvT", [C, INNER], BF16, kind="ExternalInput").ap()
    wo_d = nc.dram_tensor("woT", [INNER, C], BF16, kind="ExternalInput").ap()
    bb_d = nc.dram_tensor("bias_bc", [128, 2, NI], F32, kind="ExternalInput").ap()
    g.out_d = nc.dram_tensor("out", [B, C, RS, WCOL], BF16, kind="ExternalOutput").ap()

    with tile.TileContext(nc) as tc:
        with (
            tc.tile_pool(name="wpool", bufs=1) as wpool,
            tc.tile_pool(name="xpool", bufs=2) as xpool,
            tc.tile_pool(name="qk", bufs=2) as qkpool,
            tc.tile_pool(name="vp", bufs=2) as vpool,
            tc.tile_pool(name="ex", bufs=3) as expool,
            tc.tile_pool(name="un", bufs=4) as unpool,
            tc.tile_pool(name="at", bufs=3) as atpool,
            tc.tile_pool(name="bc", bufs=1) as bcpool,
            tc.tile_pool(name="sp", bufs=1) as spool,
            tc.tile_pool(name="ou", bufs=2) as outpool,
            tc.tile_pool(name="pproj", bufs=2, space="PSUM") as pproj,
            tc.tile_pool(name="psim", bufs=3, space="PSUM") as psim,
            tc.tile_pool(name="dram", bufs=3, space="DRAM") as dpool,
        ):
            g.qkpool, g.vpool, g.expool = qkpool, vpool, expool
            g.unpool, g.atpool, g.bcpool, g.spool = unpool, atpool, bcpool, spool
            g.outpool, g.pproj, g.psim, g.dpool = outpool, pproj, psim, dpool

            g.wq_sb = wpool.tile([128, 2, INNER], BF16, name="wq")
            nc.sync.dma_start(g.wq_sb[:], wq_d.rearrange("(kc p) m -> p kc m", p=128))
            g.wk_sb = wpool.tile([128, 2, INNER], BF16, name="wk")
            nc.sync.dma_start(g.wk_sb[:], wk_d.rearrange("(kc p) m -> p kc m", p=128))
            g.wv_sb = wpool.tile([128, 2, INNER], BF16, name="wv")
            nc.sync.dma_start(g.wv_sb[:], wv_d.rearrange("(kc p) m -> p kc m", p=128))
            g.wo_sb = wpool.tile([128, 4, C], BF16, name="wo")
            nc.sync.dma_start(g.wo_sb[:], wo_d.rearrange("(kc p) m -> p kc m", p=128))
            g.bias_bc = wpool.tile([128, 2, NI], F32, name="bb")
            nc.sync.dma_start(g.bias_bc[:], bb_d)

            # normalize/Wo queues are carried between the two bodies of one
            # For_i iteration (same block), and fully drained at iteration
            # end — loop iterations stay self-contained.
            state = {"chain": [], "wo": []}

            def _body(drain, prefetch_next=False):
                _emit_windows(g, state, x_d, s_d, xpool, phases, drain=drain,
                              prefetch_next=prefetch_next)

            if reps == 1:
                _body(True)
            elif reps < 0:
                with tc.For_i(0, -reps, 1):   # BODIES_PER_ITER bodies/iteration
                    for bi in range(BODIES_PER_ITER):
                        last = bi == BODIES_PER_ITER - 1
                        _body(last, prefetch_next=not last)
            else:
                with tc.For_i(0, reps, 1):
                    _body(True)

    if split_mw:
        _split_multiwaits(nc)
    return nc


def _load_strips(g, b, x_d, s_d, xpool):
    """Load one batch's x/skip strip. Split into window-range chunks so the
    first windows' data lands quickly after an iteration barrier."""
    nc = g.nc
    x_sb = xpool.tile([128, 2, PX], BF16, name=f"x{b}", tag="xs")
    sk_sb = xpool.tile([128, 2, PX], BF16, name=f"s{b}", tag="ss")
    xr = x_d[b].rearrange("(kc p) r w -> p kc (r w)", p=128)
    sr = s_d[b].rearrange("(kc p) r w -> p kc (r w)", p=128)
    cuts = (0, 512, 2048)
    for c0, c1 in zip(cuts[:-1], cuts[1:]):
        nc.sync.dma_start(x_sb[:, :, c0:c1], xr[:, :, c0:c1])
        nc.sync.dma_start(sk_sb[:, :, c0:c1], sr[:, :, c0:c1])
    return (x_sb, sk_sb)


def _emit_windows(g, state, x_d, s_d, xpool, phases=3, drain=True,
                  prefetch_next=False):
    """One body: all 16 (b, y) windows flat. The normalize/Wo queues defer
    their tail into the following body when drain=False (two bodies per
    For_i iteration share one block), else drain fully. With prefetch_next,
    the next body's strips are loaded mid-body (after their last consumer)."""
    nc = g.nc
    strips = state.pop("next_strips", None)
    if strips is None:
        strips = [_load_strips(g, b, x_d, s_d, xpool) for b in range(B)]
    pairs = [(b, y0) for b in range(B) for y0 in range(0, NY, 2)]

    def drain_gen(gen):
        out = None
        for out in gen:
            pass
        return out

    cur = drain_gen(_emit_projections(g, *pairs[0], strips))
    for pidx, (b, y0) in enumerate(pairs):
        if phases < 2:
            if pidx + 1 < len(pairs):
                cur = drain_gen(_emit_projections(g, *pairs[pidx + 1], strips))
            continue
        nxt_gen = (_emit_projections(g, *pairs[pidx + 1], strips)
                   if pidx + 1 < len(pairs) else None)
        nxt_result = [None]
        def filler():
            if nxt_gen is not None:
                r = next(nxt_gen, None)
                if r is not None:
                    nxt_result[0] = r
        q_p, k_p, v_p = cur
        uns = [_emit_attention(g, b, y0 + t, q_p[t], k_p[t], v_p[t], filler=filler)
               for t in range(2)]
        if nxt_gen is not None:
            r = drain_gen(nxt_gen)
            if r is not None:
                nxt_result[0] = r
            cur = nxt_result[0]
        if phases < 3:
            continue
        state["chain"].append((b, (y0, y0 + 1), uns))
        if len(state["chain"]) > 1:
            cb, cys, cuns = state["chain"].pop(0)
            state["wo"].append((cb, cys, _emit_norm_chain(g, cb, cys, cuns)))
        if len(state["wo"]) > 1:
            wb, wys, atts = state["wo"].pop(0)
            _emit_wo(g, wb, wys, atts)
        if prefetch_next:
            if pidx == 4:
                state.setdefault("next_strips", [None, None])[0] = \
                    _load_strips(g, 0, x_d, s_d, xpool)
            elif pidx == 6:
                state["next_strips"][1] = _load_strips(g, 1, x_d, s_d, xpool)
    if drain:
        for cb, cys, cuns in state["chain"]:
            state["wo"].append((cb, cys, _emit_norm_chain(g, cb, cys, cuns)))
        state["chain"] = []
        for wb, wys, atts in state["wo"]:
            _emit_wo(g, wb, wys, atts)
        state["wo"] = []


_PROGRAM = None


def _get_program():
    global _PROGRAM
    if _PROGRAM is None:
        _PROGRAM = build_program()
    return _PROGRAM


def _make_bias_bc(bo):
    # [C] -> [128 p, 2 mc, NI] broadcast along NI (channel = mc*128 + p)
    bb = np.asarray(bo, np.float32).reshape(2, 128).transpose(1, 0)
    return np.ascontiguousarray(np.broadcast_to(bb[:, :, None], (128, 2, NI)).copy())


def kernel(x, skip, Wq, Wkv, Wo, bo):
    bf = ml_dtypes.bfloat16
    x = np.asarray(x, dtype=np.float32)
    skip = np.asarray(skip, dtype=np.float32)
    wqT = np.ascontiguousarray(np.asarray(Wq, np.float32).T).astype(bf)
    wkT = np.ascontiguousarray(np.asarray(Wkv, np.float32)[:INNER].T).astype(bf)
    wvT = np.ascontiguousarray(np.asarray(Wkv, np.float32)[INNER:].T).astype(bf)
    woT = np.ascontiguousarray(np.asarray(Wo, np.float32).T).astype(bf)
    bias_bc = _make_bias_bc(bo)

    nc = _get_program()
    in_maps = []
    for c in range(N_CORES):
        r0, r1 = c * RS, (c + 1) * RS
        in_maps.append({
            "x": _to_window_major(x[:, :, r0:r1, :]).astype(bf),
            "skip": _to_window_major(skip[:, :, r0:r1, :]).astype(bf),
            "wqT": wqT, "wkT": wkT, "wvT": wvT, "woT": woT,
            "bias_bc": bias_bc,
        })
    res = run_bass_kernel_spmd(nc, in_maps, list(range(N_CORES)))
    out = np.empty((B, C, N_CORES * RS, WCOL), dtype=np.float32)
    for c in range(N_CORES):
        out[:, :, c * RS:(c + 1) * RS, :] = _from_window_major(
            np.asarray(res.results[c]["out"]).astype(np.float32))
    return out


def _to_window_major(strip):
    # [B, C, 16, 128] row-major pixels -> pixel axis reordered to (y, r, c)
    s = strip.reshape(B, C, RS, NY, WIN).transpose(0, 1, 3, 2, 4)
    return np.ascontiguousarray(s).reshape(B, C, RS, WCOL)


def _from_window_major(strip):
    # inverse of _to_window_major
    s = strip.reshape(B, C, NY, RS, WIN).transpose(0, 1, 3, 2, 4)
    return np.ascontiguousarray(s).reshape(B, C, RS, WCOL)
